# revision 27
# baseline (speedup 1.0000x reference)
"""Multi-head self-attention (ANE-style 1x1-conv attention) on 8 trn2 cores.

Sharding: (batch, head-group) tensor parallel. Core c handles batch
b = c//2 and heads [8*(c%2), 8*(c%2)+8) over the FULL sequence: q/k/v
projection weights split 512 out-features per core, out-projection
contraction split 512 in-features per core. The two cores of a batch
return partial yT sums; the host adds them (plus the bias) during the
gather -- no on-device collective, no duplicated k/v work.

Per-core pipeline (fp16 operands, fp32 PSUM accumulation):
  1. q/k proj -> resident SBUF per head-pair (128 = 2*Dh partitions).
  2. v proj emitted pre-transposed (lhsT = x chunk) so V lands as
     (l on partitions, features free) -- the AV stationary layout --
     with a ones column per head (vaug 65-wide): the AV matmul also
     accumulates the softmax denominator in PSUM row 64.
  3. attention per (pair, i-quarter): scores are TWO K=64 row-tiled
     matmuls (tile_position (0,0)/(64,0)) concurrent on the PE array;
     one exp ACT per jc covers both heads (128x1024 PSUM). AV (M=65)
     trails exp by 2 jc. ACT is the pacing engine; projection /
     out-projection psum-groups inject as PE fillers in the jc loop.
  4. normalization is a 3-stage pipeline spread across blocks so no
     engine queue ever head-blocks on a DMA roundtrip:
       stage A (block k):   osb copy (DVE), denom row -> DRAM
                            (gpsimd), packed reload (sync)
       stage B (block k+1): reciprocal (DVE), -> DRAM (gpsimd),
                            partition-broadcast reload (sync)
       stage C (block k+2): o_res = osb * recip (gpsimd)
  5. out-proj per i-quarter as fillers/tail; yT partial (fp16) to
     DRAM; host sums core pairs and adds the bias.
"""

import numpy as np

import concourse.bass as bass
import concourse.tile as tile
from concourse import bacc, mybir
from concourse.bass_utils import run_bass_kernel_spmd

B, D, L, H, Dh = 4, 1024, 2048, 16, 64
NCORES = 8
F32 = mybir.dt.float32
F16 = mybir.dt.float16
ACT_EXP = mybir.ActivationFunctionType.Exp
INV_SCALE = 1.0 / 8.0  # 1/sqrt(Dh)

NP = D // 128      # 8 x-chunks of the model dim
NPAIR = 4          # head pairs per core (8 heads)
NJC = L // 128     # 16 key chunks
NIQ = 4            # query quarters
IQ = L // NIQ      # 512 queries per quarter
NO = 512           # projected features per core (8 heads * 64)


def build_nc():
    nc = bacc.Bacc()
    x = nc.dram_tensor("x", [D, L], F16, kind="ExternalInput")
    wqT = nc.dram_tensor("wqT", [D, NO], F16, kind="ExternalInput")
    wkT = nc.dram_tensor("wkT", [D, NO], F16, kind="ExternalInput")
    wvT = nc.dram_tensor("wvT", [D, NO], F16, kind="ExternalInput")
    woT = nc.dram_tensor("woT", [NO, D], F16, kind="ExternalInput")
    ones16 = nc.dram_tensor("ones16", [1, NJC], F16, kind="ExternalInput")
    yT = nc.dram_tensor("yT", [L, D], F16, kind="ExternalOutput")

    with tile.TileContext(nc) as tc:
        with (
            nc.allow_low_precision(reason="fp16 operands by design"),
            tc.tile_pool(name="dram", bufs=1, space="DRAM") as dram,
            tc.tile_pool(name="keep", bufs=1) as keep,
            tc.tile_pool(name="attn", bufs=1) as attn,
            tc.tile_pool(name="ps", bufs=2, space="PSUM") as ps,
        ):
            # ------------- input loads -------------
            # DMA transfers serialize on one global engine pool, so only
            # completion ORDER matters: small q/k weights first, then x
            # chunks (compute chases arrivals), then v/o weights.
            wq_sb, wk_sb, wv_sb, wo_sb, x_sb = [], [], [], [], []
            for kc in range(NP):
                wt = keep.tile([128, NO], F16, name=f"wq{kc}", tag=f"wq{kc}")
                nc.sync.dma_start(out=wt, in_=wqT[128 * kc:128 * (kc + 1), :])
                wq_sb.append(wt)
            for kc in range(NP):
                wt = keep.tile([128, NO], F16, name=f"wk{kc}", tag=f"wk{kc}")
                nc.gpsimd.dma_start(out=wt,
                                    in_=wkT[128 * kc:128 * (kc + 1), :])
                wk_sb.append(wt)
            for kc in range(NP):
                xt = keep.tile([128, L], F16, name=f"x{kc}", tag=f"x{kc}")
                eng = [nc.scalar, nc.sync, nc.gpsimd][kc % 3]
                eng.dma_start(out=xt, in_=x[128 * kc:128 * (kc + 1), :])
                x_sb.append(xt)
            for kc in range(NP):
                wt = keep.tile([128, NO], F16, name=f"wv{kc}", tag=f"wv{kc}")
                nc.sync.dma_start(out=wt,
                                  in_=wvT[128 * kc:128 * (kc + 1), :])
                wv_sb.append(wt)
            for kc in range(NPAIR):
                wt = keep.tile([128, D], F16, name=f"wo{kc}", tag=f"wo{kc}")
                nc.sync.dma_start(out=wt,
                                  in_=woT[128 * kc:128 * (kc + 1), :])
                wo_sb.append(wt)

            q_res = [attn.tile([128, L], F16, name=f"q{t}", tag=f"q{t}")
                     for t in range(NPAIR)]
            k_res = [attn.tile([128, L], F16, name=f"k{t}", tag=f"k{t}")
                     for t in range(NPAIR)]
            o_res = [attn.tile([128, L], F16, name=f"o{t}", tag=f"o{t}")
                     for t in range(NPAIR)]
            # vaug: (j-part, jc, [V_he(64) | 1 | V_ho(64) | 1]) per pair
            vaug = [attn.tile([128, NJC, 130], F16, name=f"v{t}",
                              tag=f"v{t}") for t in range(NPAIR)]
            for t in range(NPAIR):
                for e in range(2):
                    nc.gpsimd.memset(vaug[t][:, :, 65 * e + 64:65 * e + 65],
                                     1.0)

            def ps_s():
                return ps.tile([128, 1024], F32, name="ps_s", tag="ps_s",
                               bufs=2)

            def ps_o():
                return ps.tile([128, IQ], F32, name="ps_o", tag="ps_o",
                               bufs=2)

            def ps_f():
                return ps.tile([128, 512], F32, name="ps_f", tag="ps_f",
                               bufs=2)

            # ------------- projection psum-group units -------------
            # each unit is split into two half-closures (the psum
            # accumulation group stays open across the split) so a PE
            # filler burst never exceeds ~0.9us between score matmuls
            def qk_unit_halves(t, n, which):
                w_sb = wq_sb if which == "q" else wk_sb
                dst = q_res[t] if which == "q" else k_res[t]
                g = {}

                def half(lo, hi):
                    def emit():
                        if lo == 0:
                            g["ps"] = ps_f()
                        for kc in range(lo, hi):
                            nc.tensor.matmul(
                                g["ps"],
                                lhsT=w_sb[kc][:, 128 * t:128 * (t + 1)],
                                rhs=x_sb[kc][:, 512 * n:512 * (n + 1)],
                                start=(kc == 0), stop=(kc == NP - 1),
                                skip_group_check=True)
                        if hi == NP:
                            nc.vector.tensor_copy(
                                out=dst[:, 512 * n:512 * (n + 1)],
                                in_=g["ps"])
                    return emit
                return [half(0, 4), half(4, NP)]

            def v_unit_halves(lc):
                # vT chunk: (128 l-parts, 512 features), lhsT = x chunk
                g = {}

                def half(lo, hi):
                    def emit():
                        if lo == 0:
                            g["ps"] = ps_f()
                        for kc in range(lo, hi):
                            nc.tensor.matmul(
                                g["ps"],
                                lhsT=x_sb[kc][:, 128 * lc:128 * (lc + 1)],
                                rhs=wv_sb[kc],
                                start=(kc == 0), stop=(kc == NP - 1),
                                skip_group_check=True)
                        if hi == NP:
                            for t in range(NPAIR):
                                src = g["ps"][:, 128 * t:128 * (t + 1)
                                              ].rearrange(
                                    "p (e c) -> p e c", e=2)
                                dst = vaug[t][:, lc].rearrange(
                                    "p (e c) -> p e c", c=65)[:, :, 0:64]
                                nc.vector.tensor_copy(out=dst, in_=src)
                    return emit
                return [half(0, 4), half(4, NP)]

            def o_unit_halves(iq, mi, n, alt=False):
                g = {}

                def half(lo, hi):
                    def emit():
                        if lo == 0:
                            if alt:
                                g["ps"] = ps.tile([128, 1024], F32,
                                                  name="ps_s", tag="ps_s",
                                                  bufs=2)[:, 0:512]
                            else:
                                g["ps"] = ps_f()
                        for kc in range(lo, hi):
                            nc.tensor.matmul(
                                g["ps"],
                                lhsT=o_res[kc][:, 512 * iq + 128 * mi:
                                               512 * iq + 128 * (mi + 1)],
                                rhs=wo_sb[kc][:, 512 * n:512 * (n + 1)],
                                start=(kc == 0), stop=(kc == NPAIR - 1),
                                skip_group_check=True)
                        if hi == NPAIR:
                            ysb = attn.tile([128, 512], F16, name="ysb",
                                            tag="ysb", bufs=4)
                            nc.vector.tensor_copy(out=ysb, in_=g["ps"])
                            eng = nc.sync if (mi + n) % 2 == 0 else nc.gpsimd
                            eng.dma_start(
                                out=yT[512 * iq + 128 * mi:
                                       512 * iq + 128 * (mi + 1),
                                       512 * n:512 * (n + 1)],
                                in_=ysb)
                    return emit
                return [half(0, 2), half(2, NPAIR)]

            # ------------- pipelined normalization stages -------------
            def norm_stage_a(t, iq, o_ps):
                osb = []
                for e in range(2):
                    ot = attn.tile([65, IQ], F16, name="osb",
                                   tag=f"osb{e}", bufs=3)
                    nc.vector.tensor_copy(out=ot, in_=o_ps[e][0:65, :])
                    osb.append(ot)
                dnd = dram.tile([2, IQ], F16, name="dnd", tag=f"dnd{t}_{iq}")
                for e in range(2):
                    nc.gpsimd.dma_start(out=dnd[e:e + 1, :],
                                        in_=osb[e][64:65, :])
                # flat reload: partition a holds den_flat[8a:8a+8] where
                # den_flat = [den_he | den_ho]
                dsc = attn.tile([128, 8], F16, name="dsc", tag="dsc",
                                bufs=3)
                nc.sync.dma_start(
                    out=dsc,
                    in_=bass.AP(tensor=dnd.tensor, offset=dnd.offset,
                                ap=[[8, 128], [1, 8]]))
                return {"t": t, "iq": iq, "osb": osb, "dsc": dsc}

            def norm_stage_b(st):
                t, iq = st["t"], st["iq"]
                rsc = attn.tile([128, 8], F16, name="rsc", tag="rsc",
                                bufs=3)
                nc.vector.reciprocal(out=rsc, in_=st["dsc"])
                # rcd flat = [1/den_he | 1/den_ho], each 512 in i-order
                rcd = dram.tile([128, 8], F16, name="rcd", tag=f"rcd{t}_{iq}")
                nc.gpsimd.dma_start(out=rcd, in_=rsc)
                rbs = []
                for e in range(2):
                    rb = attn.tile([64, IQ], F16, name="rb", tag=f"rb{e}",
                                   bufs=3)
                    nc.sync.dma_start(
                        out=rb,
                        in_=bass.AP(tensor=rcd.tensor,
                                    offset=rcd.offset + IQ * e,
                                    ap=[[0, 64], [1, IQ]]))
                    rbs.append(rb)
                st["rbs"] = rbs

            def norm_stage_c(st):
                t, iq = st["t"], st["iq"]
                for e in range(2):
                    nc.gpsimd.tensor_mul(
                        out=o_res[t][64 * e:64 * (e + 1),
                                     IQ * iq:IQ * (iq + 1)],
                        in0=st["osb"][e][0:64, :], in1=st["rbs"][e])

            # ---------------- attention block ----------------
            # the last two AV matmuls + norm stage A of a block are
            # deferred (carried) into the next block's jc=1 slot so the
            # next block's scores issue immediately at the boundary
            def emit_attn(t, iq, fillers=(), carry_in=None,
                          st_b=None, st_c=None):
                fillers = list(fillers)
                per_slot = max(1, (len(fillers) + 13) // 14)
                o_ps = [ps_o(), ps_o()]
                pts = {}

                def emit_scores(jc):
                    s_ps = ps_s()
                    for e in range(2):
                        nc.tensor.matmul(
                            s_ps[:, 512 * e:512 * (e + 1)],
                            lhsT=k_res[t][64 * e:64 * (e + 1),
                                          128 * jc:128 * (jc + 1)],
                            rhs=q_res[t][64 * e:64 * (e + 1),
                                         IQ * iq:IQ * (iq + 1)],
                            start=True, stop=True,
                            tile_position=(64 * e, 0))
                    pt = attn.tile([128, 1024], F16, name="pt", tag="pt",
                                   bufs=7)
                    nc.scalar.activation(pt, s_ps, ACT_EXP, scale=INV_SCALE)
                    pts[jc] = pt

                def emit_av(jc):
                    pt = pts.pop(jc)
                    for e in range(2):
                        nc.tensor.matmul(
                            o_ps[e][0:65, :],
                            lhsT=vaug[t][:, jc, 65 * e:65 * (e + 1)],
                            rhs=pt[:, 512 * e:512 * (e + 1)],
                            start=(jc == 0), stop=(jc == NJC - 1),
                            skip_group_check=True)

                for jc in range(NJC):
                    emit_scores(jc)
                    if jc == 2 and st_c is not None and st_c.get("st"):
                        norm_stage_c(st_c["st"])
                    if jc == 3 and carry_in is not None:
                        carry_in()
                    if jc == 6 and st_b is not None and st_b.get("st"):
                        norm_stage_b(st_b["st"])
                    if jc >= 2:
                        for _ in range(per_slot):
                            if fillers:
                                fillers.pop(0)()
                        emit_av(jc - 2)

                cs = {}

                def carry():
                    emit_av(NJC - 2)
                    emit_av(NJC - 1)
                    while fillers:
                        fillers.pop(0)()
                    cs["st"] = norm_stage_a(t, iq, o_ps)

                return carry, cs

            # ---------------- schedule (iq-major blocks) ----------------
            for h in qk_unit_halves(0, 0, "q"):
                h()
            for h in qk_unit_halves(0, 0, "k"):
                h()

            def vg(lc):
                return v_unit_halves(lc)

            def qg(t, n):
                return qk_unit_halves(t, n, "q")

            def kg(t, n):
                return qk_unit_halves(t, n, "k")

            def o_units(iq, lo, hi, alt=False):
                return sum((o_unit_halves(iq, mi, n,
                                          alt=(alt and (mi + n) % 2 == 1))
                            for mi in range(4) for n in range(2)),
                           [])[2 * lo:2 * hi]

            fillers_by_slot = {
                (0, 0): (vg(0) + vg(1) + kg(0, 1) + vg(2) + vg(3)
                         + kg(0, 2) + vg(4) + vg(5) + kg(0, 3)
                         + sum((vg(lc) for lc in range(6, 14)), [])
                         + qg(1, 0) + kg(1, 0) + vg(14) + vg(15)),
                (0, 1): kg(1, 1) + kg(1, 2) + kg(1, 3) + qg(2, 0) + kg(2, 0),
                (0, 2): kg(2, 1) + kg(2, 2) + kg(2, 3) + qg(3, 0) + kg(3, 0),
                (0, 3): kg(3, 1) + kg(3, 2) + kg(3, 3) + qg(0, 1),
                (1, 0): qg(1, 1) + qg(2, 1) + qg(3, 1),
                (1, 1): qg(0, 2) + qg(1, 2) + qg(2, 2),
                (1, 2): o_units(0, 0, 4),
                (1, 3): o_units(0, 4, 8),
                (2, 0): qg(3, 2) + qg(0, 3) + qg(1, 3),
                (2, 1): qg(2, 3) + qg(3, 3),
                (2, 2): o_units(1, 0, 4),
                (2, 3): o_units(1, 4, 8),
                (3, 2): o_units(2, 0, 4),
                (3, 3): o_units(2, 4, 8),
            }
            carry, hist = None, []
            for iq in range(NIQ):
                for t in range(NPAIR):
                    carry, cs = emit_attn(
                        t, iq, fillers_by_slot.get((iq, t), ()),
                        carry_in=carry,
                        st_b=hist[-1] if len(hist) >= 1 else None,
                        st_c=hist[-2] if len(hist) >= 2 else None)
                    hist.append(cs)
            carry()  # block 15: AV tail + stage A
            norm_stage_c(hist[-2]["st"])   # C(14)
            norm_stage_b(hist[-1]["st"])   # B(15)
            norm_stage_c(hist[-1]["st"])   # C(15)
            for h in o_units(3, 0, 8):
                h()

    nc.compile()
    return nc


_NC_CACHE = []


def kernel_with_results(x, wq, wk, wv, wo, bo, **run_kwargs):
    x = np.asarray(x, dtype=np.float32)
    wqT = np.asarray(wq, dtype=np.float32).T.astype(np.float16)
    wkT = np.asarray(wk, dtype=np.float32).T.astype(np.float16)
    wvT = np.asarray(wv, dtype=np.float32).T.astype(np.float16)
    woT = np.asarray(wo, dtype=np.float32).T.astype(np.float16)
    bo = np.asarray(bo, dtype=np.float32)

    if not _NC_CACHE:
        _NC_CACHE.append(build_nc())
    nc = _NC_CACHE[0]

    in_maps = []
    for c in range(NCORES):
        b, hg = divmod(c, 2)
        xb = np.ascontiguousarray(x[b, :, 0, :]).astype(np.float16)
        sl = slice(NO * hg, NO * (hg + 1))
        in_maps.append({
            "x": xb,
            "wqT": np.ascontiguousarray(wqT[:, sl]),
            "wkT": np.ascontiguousarray(wkT[:, sl]),
            "wvT": np.ascontiguousarray(wvT[:, sl]),
            "woT": np.ascontiguousarray(woT[sl, :]),
            "ones16": np.ones((1, NJC), dtype=np.float16),
        })

    kres = run_bass_kernel_spmd(nc, in_maps, list(range(NCORES)), **run_kwargs)

    out = np.empty((B, D, 1, L), dtype=np.float32)
    for b in range(B):
        yT0 = kres.results[2 * b]["yT"].astype(np.float32)
        yT1 = kres.results[2 * b + 1]["yT"].astype(np.float32)
        out[b, :, 0, :] = (yT0 + yT1).T + bo[:, None]
    return out, kres


def kernel(x, wq, wk, wv, wo, bo):
    out, _ = kernel_with_results(x, wq, wk, wv, wo, bo)
    return out
